# revision 2
# baseline (speedup 1.0000x reference)
"""Causal attention (DS_FullAttention) Trainium2 Bass kernel, v2.

Problem: B=4, H=8, L=S=2048, E=64 causal attention with a per-batch
exp(tau) de-stationarization rescale, fp32 I/O.

Sharding: 32 (b, h) pairs over 8 cores, 4 pairs/core. Per pair:
  - scoresT[s, q] = K^T Q computed transposed (s on PSUM partitions);
    consecutive s-chunks alternate 64-row PE tiles (row groups 0-63 /
    64-127, q/k duplicated across both halves) so chunk pairs run
    concurrently in the split systolic array.
  - softmax exp is SPLIT across two engines: the scalar engine (native
    exp ACTIVATE, scale folded in) and the vector engine (one-op
    Schraudolph: at16 = bitcast(int16(round(s*A' + B'))), A' folding
    scale*log2e*1024, B' = 15360-44; max mult err ~3%, which softmax
    normalization largely cancels).
  - causal masking: block-level (skip s-chunks above the diagonal) plus
    a triangular fp16 mask multiply on diagonal chunks, on GPSIMD (its
    tensor_tensor ext-isa lib is pre-warmed; keeps DVE free for exp).
  - V carries a ones column LAST (index 64), so A @ V also produces
    softmax denominators in PSUM partition 64.
  - normalization: reciprocal_approx_fast over PSUM rows 0:65 (custom
    DVE ops require base partition 0; rows 0:63 are discarded), row 64
    round-trips through DRAM with a stride-0 read to broadcast across
    partitions 0:64, one tensor_tensor multiply, DMA out.

Matmuls run in fp16 (hides PE weight loads; fp32 cannot).
"""

import sys

if "/opt/trn_rl_repo" not in sys.path:
    sys.path.insert(0, "/opt/trn_rl_repo")

import numpy as np

import concourse.bass as bass
import concourse.mybir as mybir
import concourse.tile as tile
from concourse import bacc, bass_utils
from concourse.bass import broadcast_tensor_aps

B, L, S, H, E = 4, 2048, 2048, 8, 64
P = 128
NCORES = 8
PAIRS_PER_CORE = (B * H) // NCORES  # 4
NQB = L // 512  # 4 q-superblocks of 512
NSC = S // P  # 16 s-chunks of 128
E2 = E + 1  # V plus ones column (ones LAST: denom at row 64)
EXP_GROUP = 2  # s-chunks exp'd per instruction (2 PSUM banks)

# Schraudolph constant: round(s*A' + B') bit-cast to fp16.
B_IMM = 15360.0 - 44.0
LOG2E_1024 = float(np.log2(np.e)) * 1024.0

# route every (cnt % 8)-th group in this set to the DVE Schraudolph exp
DVE_SLOTS = (1, 4, 6)

f32 = mybir.dt.float32
fp16 = mybir.dt.float16
i16 = mybir.dt.int16
Exp = mybir.ActivationFunctionType.Exp
Mult = mybir.AluOpType.mult
Add = mybir.AluOpType.add

_PROGRAM_CACHE = {}


def _build_program():
    if "nc" in _PROGRAM_CACHE:
        return _PROGRAM_CACHE["nc"]

    nc = bacc.Bacc(
        "TRN2",
        target_bir_lowering=False,
        debug=False,
        enable_asserts=False,
        num_devices=NCORES,
    )
    qt_d = nc.dram_tensor("qt", [PAIRS_PER_CORE, P, L], fp16, kind="ExternalInput")
    kt_d = nc.dram_tensor("kt", [PAIRS_PER_CORE, P, L], fp16, kind="ExternalInput")
    vp_d = nc.dram_tensor(
        "vp", [PAIRS_PER_CORE, P, NSC, E2], fp16, kind="ExternalInput"
    )
    tri_d = nc.dram_tensor("tri", [P, P], fp16, kind="ExternalInput")
    scl_d = nc.dram_tensor("scl", [P, PAIRS_PER_CORE], f32, kind="ExternalInput")
    sclA_d = nc.dram_tensor("sclA", [P, PAIRS_PER_CORE], f32, kind="ExternalInput")
    o_d = nc.dram_tensor("o", [PAIRS_PER_CORE, E, L], f32, kind="ExternalOutput")

    with tile.TileContext(nc) as tc:
        with (
            tc.tile_pool(name="const", bufs=1) as const,
            tc.tile_pool(name="qk", bufs=2) as qk,
            tc.tile_pool(name="atp", bufs=5) as atp,
            tc.tile_pool(name="stg", bufs=3) as stg,
            tc.tile_pool(name="psS", bufs=3, space="PSUM") as psS,
            tc.tile_pool(name="psO", bufs=2, space="PSUM") as psO,
            tc.tile_pool(name="dram", bufs=3, space="DRAM") as dram,
        ):
            # warm-up exp: pulls the ~2.7us ACT table load under the DMAs
            wu = const.tile([P, 16], f32)
            nc.gpsimd.memset(wu[:], 0.0)
            nc.scalar.activation(wu[:], wu[:], Exp, scale=1.0)
            # warm the GPSIMD tensor_tensor ext-isa lib (~6us IRAM load)
            wg = const.tile([P, 16], fp16)
            nc.gpsimd.memset(wg[:], 1.0)
            nc.gpsimd.tensor_tensor(wg[:], wg[:], wg[:], Mult)

            tri_t = const.tile([P, P], fp16)
            nc.sync.dma_start(tri_t[:], tri_d[:])
            scl_t = const.tile([P, PAIRS_PER_CORE], f32)
            nc.sync.dma_start(scl_t[:], scl_d[:])
            sclA_t = const.tile([P, PAIRS_PER_CORE], f32)
            nc.sync.dma_start(sclA_t[:], sclA_d[:])

            # mm2 + epilogues run one exp-group behind (RAW decoupling)
            pending = []

            def flush(depth=0):
                while len(pending) > depth:
                    pending.pop(0)()

            group_cnt = [0]

            for p in range(PAIRS_PER_CORE):
                qt_t = qk.tile([P, L], fp16, tag="qt")
                kt_t = qk.tile([P, L], fp16, tag="kt")
                vp_t = qk.tile([P, NSC, E2], fp16, tag="vp")
                if p == 0:  # small first slices so group 0 starts early
                    nc.sync.dma_start(kt_t[:, 0:512], kt_d[p][:, 0:512])
                    nc.sync.dma_start(qt_t[:, 0:512], qt_d[p][:, 0:512])
                    nc.sync.dma_start(vp_t[:, 0:4, :], vp_d[p][:, 0:4, :])
                    nc.sync.dma_start(kt_t[:, 512:L], kt_d[p][:, 512:L])
                    nc.sync.dma_start(qt_t[:, 512:L], qt_d[p][:, 512:L])
                    nc.sync.dma_start(vp_t[:, 4:NSC, :], vp_d[p][:, 4:NSC, :])
                else:
                    nc.sync.dma_start(qt_t[:], qt_d[p])
                    nc.sync.dma_start(kt_t[:], kt_d[p])
                    nc.sync.dma_start(vp_t[:], vp_d[p])

                for iq in range(NQB):
                    q0 = 512 * iq
                    njs = 4 * iq + 4  # s-chunks this q-superblock touches
                    po = psO.tile([P, 512], f32, tag="po")

                    def mk_mm2(js, at, po=po, vp_t=vp_t, iq=iq, njs=njs,
                               p=p, q0=q0):
                        def emit():
                            for idx, j in enumerate(js):
                                qoff = max(0, P * (j - 4 * iq))
                                nc.tensor.matmul(
                                    po[0:E2, qoff:512],
                                    lhsT=vp_t[:, j, :],
                                    rhs=at[:, idx, qoff:512],
                                    start=(j == 0),
                                    stop=(j == njs - 1),
                                )
                            if js[-1] != njs - 1:
                                return
                            # q-superblock epilogue: invert denominators
                            # (rows 0:63 are garbage, custom-DVE ops need
                            # base partition 0), broadcast row 64 via a
                            # DRAM round trip, normalize, store.
                            rr = stg.tile([P, 512], f32, tag="rr")
                            nc.vector.reciprocal_approx_fast(
                                rr[0:E2, :], po[0:E2, :]
                            )
                            rd = dram.tile([1, 512], f32)
                            nc.sync.dma_start(rd[:], rr[64:65, :])
                            r64 = stg.tile([P, 512], f32, tag="r64")
                            bsrc, bdst = broadcast_tensor_aps(
                                rd[:], r64[0:E, :]
                            )
                            nc.sync.dma_start(bdst, bsrc)
                            outF = stg.tile([P, 512], f32, tag="outF")
                            nc.vector.tensor_tensor(
                                outF[0:E, :], po[0:E, :], r64[0:E, :], Mult
                            )
                            nc.sync.dma_start(
                                o_d[p, :, q0 : q0 + 512], outF[0:E, :]
                            )

                        return emit

                    for g0 in range(0, njs, EXP_GROUP):
                        js = [g0, g0 + 1]
                        ps = psS.tile([P, EXP_GROUP, 512], f32, tag="ps")
                        for idx, j in enumerate(js):
                            row = 64 * (j % 2)  # alternate 64-row PE tiles
                            qoff = max(0, P * (j - 4 * iq))
                            nc.tensor.matmul(
                                ps[:, idx, qoff:512],
                                lhsT=kt_t[row : row + 64, P * j : P * (j + 1)],
                                rhs=qt_t[row : row + 64, q0 + qoff : q0 + 512],
                                start=True,
                                stop=True,
                            )
                        at = atp.tile([P, EXP_GROUP, 512], fp16, tag="at")
                        qmin = min(max(0, P * (j - 4 * iq)) for j in js)
                        if (group_cnt[0] % 8) in DVE_SLOTS:
                            nc.vector.tensor_scalar(
                                at[:, :, qmin:512].bitcast(i16),
                                ps[:, :, qmin:512],
                                sclA_t[:, p : p + 1],
                                B_IMM,
                                Mult,
                                Add,
                            )
                        else:
                            nc.scalar.activation(
                                at[:, :, qmin:512],
                                ps[:, :, qmin:512],
                                Exp,
                                scale=scl_t[:, p : p + 1],
                            )
                        group_cnt[0] += 1
                        for idx, j in enumerate(js):
                            d = j - 4 * iq
                            if d >= 0:  # diagonal chunk: triangular mask
                                qo = P * d
                                nc.gpsimd.tensor_tensor(
                                    at[:, idx, qo : qo + P],
                                    at[:, idx, qo : qo + P],
                                    tri_t[:],
                                    Mult,
                                )
                        flush(depth=1)
                        pending.append(mk_mm2(js, at))
            flush()

    nc.compile()
    _PROGRAM_CACHE["nc"] = nc
    return nc


def _prep_core_inputs(queries, keys, values, tau, core):
    qt = np.empty((PAIRS_PER_CORE, P, L), dtype=np.float16)
    kt = np.empty((PAIRS_PER_CORE, P, L), dtype=np.float16)
    vp = np.zeros((PAIRS_PER_CORE, P, NSC, E2), dtype=np.float16)
    scl = np.empty((P, PAIRS_PER_CORE), dtype=np.float32)
    sclA = np.empty((P, PAIRS_PER_CORE), dtype=np.float32)
    for p in range(PAIRS_PER_CORE):
        idx = PAIRS_PER_CORE * core + p
        b, h = divmod(idx, H)
        qT = np.ascontiguousarray(queries[b, :, h, :].T).astype(np.float16)  # [E, L]
        kT = np.ascontiguousarray(keys[b, :, h, :].T).astype(np.float16)
        qt[p, 0:E] = qT
        qt[p, E:P] = qT
        kt[p, 0:E] = kT
        kt[p, E:P] = kT
        # vp[p, si, so, e] = V[b, 128*so + si, h, e]; ones in column E (last)
        vv = values[b, :, h, :].reshape(NSC, P, E).transpose(1, 0, 2)
        vp[p, :, :, 0:E] = vv.astype(np.float16)
        vp[p, :, :, E] = 1.0
        c = np.exp(tau[b, 0, 0, 0]) / np.sqrt(E)
        scl[:, p] = c
        sclA[:, p] = c * LOG2E_1024
    tri = np.triu(np.ones((P, P), dtype=np.float16))  # tri[s, q] = 1 iff s <= q
    return {"qt": qt, "kt": kt, "vp": vp, "tri": tri, "scl": scl, "sclA": sclA}


def _run(inputs, trace=False):
    queries = np.asarray(inputs["queries"], dtype=np.float32)
    keys = np.asarray(inputs["keys"], dtype=np.float32)
    values = np.asarray(inputs["values"], dtype=np.float32)
    tau = np.asarray(inputs["tau"], dtype=np.float32)

    nc = _build_program()
    in_maps = [
        _prep_core_inputs(queries, keys, values, tau, c) for c in range(NCORES)
    ]
    res = bass_utils.run_bass_kernel_spmd(
        nc, in_maps, core_ids=list(range(NCORES)), trace=trace
    )
    out = np.empty((B, L, H, E), dtype=np.float32)
    for c in range(NCORES):
        o = res.results[c]["o"]  # [PAIRS, E, L]
        for p in range(PAIRS_PER_CORE):
            idx = PAIRS_PER_CORE * c + p
            b, h = divmod(idx, H)
            out[b, :, h, :] = o[p].T
    return out, res


def kernel(queries, keys, values, attn_mask, tau):
    out, _ = _run(
        {"queries": queries, "keys": keys, "values": values, "tau": tau},
        trace=False,
    )
    return out


def kernel_traced(queries, keys, values, attn_mask, tau):
    out, res = _run(
        {"queries": queries, "keys": keys, "values": values, "tau": tau},
        trace=True,
    )
    return out, res


# revision 3
# speedup vs baseline: 1.0112x; 1.0112x over previous
"""Causal attention (DS_FullAttention) Trainium2 Bass kernel, v2.

Problem: B=4, H=8, L=S=2048, E=64 causal attention with a per-batch
exp(tau) de-stationarization rescale, fp32 I/O.

Sharding: 32 (b, h) pairs over 8 cores, 4 pairs/core. Per pair:
  - scoresT[s, q] = K^T Q computed transposed (s on PSUM partitions);
    consecutive s-chunks alternate 64-row PE tiles (row groups 0-63 /
    64-127, q/k duplicated across both halves) so chunk pairs run
    concurrently in the split systolic array.
  - softmax exp is SPLIT across two engines: the scalar engine (native
    exp ACTIVATE, scale folded in) and the vector engine (one-op
    Schraudolph: at16 = bitcast(int16(round(s*A' + B'))), A' folding
    scale*log2e*1024, B' = 15360-44; max mult err ~3%, which softmax
    normalization largely cancels).
  - causal masking: block-level (skip s-chunks above the diagonal) plus
    a triangular fp16 mask multiply on diagonal chunks, on GPSIMD (its
    tensor_tensor ext-isa lib is pre-warmed; keeps DVE free for exp).
  - V carries a ones column LAST (index 64), so A @ V also produces
    softmax denominators in PSUM partition 64.
  - normalization: reciprocal_approx_fast over PSUM rows 0:65 (custom
    DVE ops require base partition 0; rows 0:63 are discarded), row 64
    round-trips through DRAM with a stride-0 read to broadcast across
    partitions 0:64, one tensor_tensor multiply, DMA out.

Matmuls run in fp16 (hides PE weight loads; fp32 cannot).
"""

import sys

if "/opt/trn_rl_repo" not in sys.path:
    sys.path.insert(0, "/opt/trn_rl_repo")

import numpy as np

import concourse.bass as bass
import concourse.mybir as mybir
import concourse.tile as tile
from concourse import bacc, bass_utils
from concourse.bass import broadcast_tensor_aps

B, L, S, H, E = 4, 2048, 2048, 8, 64
P = 128
NCORES = 8
PAIRS_PER_CORE = (B * H) // NCORES  # 4
NQB = L // 512  # 4 q-superblocks of 512
NSC = S // P  # 16 s-chunks of 128
E2 = E + 1  # V plus ones column (ones LAST: denom at row 64)
EXP_GROUP = 2  # s-chunks exp'd per instruction (2 PSUM banks)

# Schraudolph constant: round(s*A' + B') bit-cast to fp16.
B_IMM = 15360.0 - 44.0
LOG2E_1024 = float(np.log2(np.e)) * 1024.0

# route every (cnt % 8)-th group in this set to the DVE Schraudolph exp
DVE_SLOTS = (1, 4, 6)

f32 = mybir.dt.float32
fp16 = mybir.dt.float16
i16 = mybir.dt.int16
Exp = mybir.ActivationFunctionType.Exp
Mult = mybir.AluOpType.mult
Add = mybir.AluOpType.add

_PROGRAM_CACHE = {}


def _build_program():
    if "nc" in _PROGRAM_CACHE:
        return _PROGRAM_CACHE["nc"]

    nc = bacc.Bacc(
        "TRN2",
        target_bir_lowering=False,
        debug=False,
        enable_asserts=False,
        num_devices=NCORES,
    )
    qt_d = nc.dram_tensor("qt", [PAIRS_PER_CORE, P, L], fp16, kind="ExternalInput")
    kt_d = nc.dram_tensor("kt", [PAIRS_PER_CORE, P, L], fp16, kind="ExternalInput")
    vp_d = nc.dram_tensor(
        "vp", [PAIRS_PER_CORE, P, NSC, E2], fp16, kind="ExternalInput"
    )
    tri_d = nc.dram_tensor("tri", [P, P], fp16, kind="ExternalInput")
    scl_d = nc.dram_tensor("scl", [P, PAIRS_PER_CORE], f32, kind="ExternalInput")
    sclA_d = nc.dram_tensor("sclA", [P, PAIRS_PER_CORE], f32, kind="ExternalInput")
    o_d = nc.dram_tensor("o", [PAIRS_PER_CORE, E, L], f32, kind="ExternalOutput")

    with tile.TileContext(nc) as tc:
        with (
            tc.tile_pool(name="const", bufs=1) as const,
            tc.tile_pool(name="qk", bufs=2) as qk,
            tc.tile_pool(name="atp", bufs=6) as atp,
            tc.tile_pool(name="stg", bufs=4) as stg,
            tc.tile_pool(name="psS", bufs=3, space="PSUM") as psS,
            tc.tile_pool(name="psO", bufs=2, space="PSUM") as psO,
            tc.tile_pool(name="dram", bufs=3, space="DRAM") as dram,
        ):
            # warm-up exp: pulls the ~2.7us ACT table load under the DMAs
            wu = const.tile([P, 16], f32)
            nc.gpsimd.memset(wu[:], 0.0)
            nc.scalar.activation(wu[:], wu[:], Exp, scale=1.0)
            # warm the GPSIMD tensor_tensor ext-isa lib (~6us IRAM load)
            wg = const.tile([P, 16], fp16)
            nc.gpsimd.memset(wg[:], 1.0)
            nc.gpsimd.tensor_tensor(wg[:], wg[:], wg[:], Mult)

            tri_t = const.tile([P, P], fp16)
            scl_t = const.tile([P, PAIRS_PER_CORE], f32)
            sclA_t = const.tile([P, PAIRS_PER_CORE], f32)

            # mm2 + epilogues run two exp-groups behind (RAW decoupling);
            # epilogue mult+store defer further so the DRAM-broadcast round
            # trip never blocks the DVE queue head.
            pending = []
            pending_fin = []

            def flush(depth=0):
                while len(pending) > depth:
                    pending.pop(0)()

            def flush_fin(depth=0):
                while len(pending_fin) > depth:
                    pending_fin.pop(0)()

            group_cnt = [0]

            for p in range(PAIRS_PER_CORE):
                qt_t = qk.tile([P, L], fp16, tag="qt")
                kt_t = qk.tile([P, L], fp16, tag="kt")
                vp_t = qk.tile([P, NSC, E2], fp16, tag="vp")
                if p == 0:  # small first slices so group 0 starts early
                    nc.sync.dma_start(kt_t[:, 0:512], kt_d[p][:, 0:512])
                    nc.sync.dma_start(qt_t[:, 0:512], qt_d[p][:, 0:512])
                    nc.sync.dma_start(scl_t[:], scl_d[:])
                    nc.sync.dma_start(sclA_t[:], sclA_d[:])
                    nc.sync.dma_start(tri_t[:], tri_d[:])
                    nc.sync.dma_start(vp_t[:, 0:4, :], vp_d[p][:, 0:4, :])
                    nc.sync.dma_start(kt_t[:, 512:L], kt_d[p][:, 512:L])
                    nc.sync.dma_start(qt_t[:, 512:L], qt_d[p][:, 512:L])
                    nc.sync.dma_start(vp_t[:, 4:NSC, :], vp_d[p][:, 4:NSC, :])
                else:
                    nc.sync.dma_start(qt_t[:], qt_d[p])
                    nc.sync.dma_start(kt_t[:], kt_d[p])
                    nc.sync.dma_start(vp_t[:], vp_d[p])

                for iq in range(NQB):
                    q0 = 512 * iq
                    njs = 4 * iq + 4  # s-chunks this q-superblock touches
                    po = psO.tile([P, 512], f32, tag="po")

                    def mk_mm2(js, at, po=po, vp_t=vp_t, iq=iq, njs=njs,
                               p=p, q0=q0):
                        def emit():
                            for idx, j in enumerate(js):
                                qoff = max(0, P * (j - 4 * iq))
                                nc.tensor.matmul(
                                    po[0:E2, qoff:512],
                                    lhsT=vp_t[:, j, :],
                                    rhs=at[:, idx, qoff:512],
                                    start=(j == 0),
                                    stop=(j == njs - 1),
                                )
                            if js[-1] != njs - 1:
                                return
                            # q-superblock epilogue stage A: invert
                            # denominators (rows 0:63 are garbage,
                            # custom-DVE ops need base partition 0) and
                            # kick off the DRAM-round-trip broadcast of
                            # row 64 across partitions 0:64.
                            rr = stg.tile([P, 512], f32, tag="rr")
                            nc.vector.reciprocal_approx_fast(
                                rr[0:E2, :], po[0:E2, :]
                            )
                            rd = dram.tile([1, 512], f32)
                            nc.sync.dma_start(rd[:], rr[64:65, :])
                            r64 = stg.tile([P, 512], f32, tag="r64")
                            bsrc, bdst = broadcast_tensor_aps(
                                rd[:], r64[0:E, :]
                            )
                            nc.sync.dma_start(bdst, bsrc)

                            def fin(po=po, r64=r64, p=p, q0=q0):
                                outF = stg.tile([P, 512], f32, tag="outF")
                                nc.vector.tensor_tensor(
                                    outF[0:E, :], po[0:E, :], r64[0:E, :],
                                    Mult,
                                )
                                nc.sync.dma_start(
                                    o_d[p, :, q0 : q0 + 512], outF[0:E, :]
                                )

                            pending_fin.append(fin)

                        return emit

                    for g0 in range(0, njs, EXP_GROUP):
                        js = [g0, g0 + 1]
                        ps = psS.tile([P, EXP_GROUP, 512], f32, tag="ps")
                        for idx, j in enumerate(js):
                            row = 64 * (j % 2)  # alternate 64-row PE tiles
                            qoff = max(0, P * (j - 4 * iq))
                            nc.tensor.matmul(
                                ps[:, idx, qoff:512],
                                lhsT=kt_t[row : row + 64, P * j : P * (j + 1)],
                                rhs=qt_t[row : row + 64, q0 + qoff : q0 + 512],
                                start=True,
                                stop=True,
                            )
                        at = atp.tile([P, EXP_GROUP, 512], fp16, tag="at")
                        qmin = min(max(0, P * (j - 4 * iq)) for j in js)
                        if (group_cnt[0] % 8) in DVE_SLOTS:
                            nc.vector.tensor_scalar(
                                at[:, :, qmin:512].bitcast(i16),
                                ps[:, :, qmin:512],
                                sclA_t[:, p : p + 1],
                                B_IMM,
                                Mult,
                                Add,
                            )
                        else:
                            nc.scalar.activation(
                                at[:, :, qmin:512],
                                ps[:, :, qmin:512],
                                Exp,
                                scale=scl_t[:, p : p + 1],
                            )
                        group_cnt[0] += 1
                        for idx, j in enumerate(js):
                            d = j - 4 * iq
                            if d >= 0:  # diagonal chunk: triangular mask
                                qo = P * d
                                nc.gpsimd.tensor_tensor(
                                    at[:, idx, qo : qo + P],
                                    at[:, idx, qo : qo + P],
                                    tri_t[:],
                                    Mult,
                                )
                        flush(depth=2)
                        flush_fin(depth=1)
                        pending.append(mk_mm2(js, at))
            flush()
            flush_fin()

    nc.compile()
    _PROGRAM_CACHE["nc"] = nc
    return nc


def _prep_core_inputs(queries, keys, values, tau, core):
    qt = np.empty((PAIRS_PER_CORE, P, L), dtype=np.float16)
    kt = np.empty((PAIRS_PER_CORE, P, L), dtype=np.float16)
    vp = np.zeros((PAIRS_PER_CORE, P, NSC, E2), dtype=np.float16)
    scl = np.empty((P, PAIRS_PER_CORE), dtype=np.float32)
    sclA = np.empty((P, PAIRS_PER_CORE), dtype=np.float32)
    for p in range(PAIRS_PER_CORE):
        idx = PAIRS_PER_CORE * core + p
        b, h = divmod(idx, H)
        qT = np.ascontiguousarray(queries[b, :, h, :].T).astype(np.float16)  # [E, L]
        kT = np.ascontiguousarray(keys[b, :, h, :].T).astype(np.float16)
        qt[p, 0:E] = qT
        qt[p, E:P] = qT
        kt[p, 0:E] = kT
        kt[p, E:P] = kT
        # vp[p, si, so, e] = V[b, 128*so + si, h, e]; ones in column E (last)
        vv = values[b, :, h, :].reshape(NSC, P, E).transpose(1, 0, 2)
        vp[p, :, :, 0:E] = vv.astype(np.float16)
        vp[p, :, :, E] = 1.0
        c = np.exp(tau[b, 0, 0, 0]) / np.sqrt(E)
        scl[:, p] = c
        sclA[:, p] = c * LOG2E_1024
    tri = np.triu(np.ones((P, P), dtype=np.float16))  # tri[s, q] = 1 iff s <= q
    return {"qt": qt, "kt": kt, "vp": vp, "tri": tri, "scl": scl, "sclA": sclA}


def _run(inputs, trace=False):
    queries = np.asarray(inputs["queries"], dtype=np.float32)
    keys = np.asarray(inputs["keys"], dtype=np.float32)
    values = np.asarray(inputs["values"], dtype=np.float32)
    tau = np.asarray(inputs["tau"], dtype=np.float32)

    nc = _build_program()
    in_maps = [
        _prep_core_inputs(queries, keys, values, tau, c) for c in range(NCORES)
    ]
    res = bass_utils.run_bass_kernel_spmd(
        nc, in_maps, core_ids=list(range(NCORES)), trace=trace
    )
    out = np.empty((B, L, H, E), dtype=np.float32)
    for c in range(NCORES):
        o = res.results[c]["o"]  # [PAIRS, E, L]
        for p in range(PAIRS_PER_CORE):
            idx = PAIRS_PER_CORE * c + p
            b, h = divmod(idx, H)
            out[b, :, h, :] = o[p].T
    return out, res


def kernel(queries, keys, values, attn_mask, tau):
    out, _ = _run(
        {"queries": queries, "keys": keys, "values": values, "tau": tau},
        trace=False,
    )
    return out


def kernel_traced(queries, keys, values, attn_mask, tau):
    out, res = _run(
        {"queries": queries, "keys": keys, "values": values, "tau": tau},
        trace=True,
    )
    return out, res
